# revision 37
# baseline (speedup 1.0000x reference)
"""ANI-style AEV computer (radial + angular) on 8 Trainium2 NeuronCores.

Strategy
--------
Data-parallel over molecules (32/core), with host-side *indexing only*
(neighborlists / triple lists / one-hot bin matrices); every floating-point
operation of the AEV math runs on-device.

Angular part: the all-triples tensor is ~94% zeros under the Rca=3.5 cutoff,
so the host enumerates surviving triples (center i, neighbors j<k) into a
flat per-core list, sorted by (molecule-slot, center-half, center, species
-pair-bin).  The device computes, per 128-triple chunk:
  geometry (vectors, d^2, dot) -> 1/d and d via ACT ln/exp -> cos/sin of the
  angle -> cutoff poly -> f2 = exp(-eta (davg-shf)^2) -> f1 = q^zeta via
  exp(zeta*ln q) -> G = w*f2 (x) f1  [bf16, 32 features]
and bins G into (center, species-pair) segments with a PE matmul against a
one-hot segment matrix (PSUM-accumulated across a segment-group's chunks).

Radial part: dense over all (i,j) pairs, species-binned with a small
block-diagonal one-hot matmul.

Only one ACT table set is used (natural_log_exp): cutoff cosines are
evaluated as a degree-4 Chebyshev polynomial in u^2 (error ~1e-6), which
keeps the Activation engine free of table switches.
"""

import os
import sys

import numpy as np

for _p in ("/opt/trn_rl_repo", "/root/.axon_site/_ro/trn_rl_repo"):
    if os.path.isdir(_p) and _p not in sys.path:
        sys.path.insert(0, _p)

import concourse.bass as bass
import concourse.mybir as mybir
from concourse import bacc, tile
from concourse.bass_utils import run_bass_kernel_spmd

import ml_dtypes

AF = mybir.ActivationFunctionType
ALU = mybir.AluOpType
dt = mybir.dt
AP = bass.AP

# ---- hyperparameters (match reference) ----
NCORES = 8
M, A = 256, 24
MLOC = M // NCORES          # 32 molecules per core
RCR, RCA = 5.2, 3.5
ETA_R, ETA_A, ZETA = 16.0, 8.0, 32.0
SHF_R = np.linspace(0.9, 5.2, 17)[:-1].astype(np.float64)   # 16
SHF_A = np.linspace(0.9, 3.5, 5)[:-1].astype(np.float64)    # 4
SHF_Z = (np.arange(8) + 0.5) * np.pi / 8.0                   # 8
NPAIR, RSUB, ASUB = 10, 16, 32
NSEG = 120                  # segments per psum group = 12 centers x 10 bins
GSEG = 128                  # one-hot width (8 pad cols -> FWL weight loads)
NG = 2 * MLOC               # 64 groups/core (2 per molecule slot)
NBLK = 1                    # angular emission blocks
PGRP = 16                   # psum groups packed per PSUM bank tile
RGRP = MLOC // 4            # 8 radial groups of 4 molecules (96 = 4*24 rows)

_TRIU = np.zeros((4, 4), np.int64)
_s1, _s2 = np.triu_indices(4)
_TRIU[_s1, _s2] = np.arange(len(_s1))
_TRIU[_s2, _s1] = _TRIU[_s1, _s2]

# ---- degree-4 (in v=u^2) Chebyshev fit of cos(pi*u/2) on u in [0,1] ----
def _cos_poly():
    v = np.linspace(0.0, 1.0, 4001)
    tgt = np.cos(0.5 * np.pi * np.sqrt(v))
    from numpy.polynomial import chebyshev as C
    ch = C.Chebyshev.fit(v, tgt, 4, domain=[0, 1])
    pw = ch.convert(kind=np.polynomial.Polynomial)
    c = pw.coef  # c0..c4 in v
    K = c[4]
    a = c[:4] / K  # monic residual coeffs a0..a3
    err = np.abs(np.polyval(c[::-1], v) - tgt).max()
    return K, a, err

_POLY_K, _POLY_A, _POLY_ERR = _cos_poly()

# const tile column map ([128, 60] fp32)
_C_SHF2A = 0     # 4  : 2*shf_a
_C_SHFR = 4      # 16 : shf_r
_C_CZH = 20      # 8  : 0.5*cos(shf_z)
_C_SZH = 28      # 8  : 0.5*sin(shf_z)
_C_MASK = 36     # 24 : radial i==j mask*100 (valid on partitions 0..95)
_C_F2B = 60      # 1  : angular exp bias ln(2*K^4)
_C_RADB = 61     # 1  : radial exp bias ln(0.25*K^2)
_C_W = 62


def _build_consts():
    ct = np.zeros((128, _C_W), np.float32)
    ct[:, _C_SHF2A:_C_SHF2A + 4] = 2.0 * SHF_A
    ct[:, _C_SHFR:_C_SHFR + 16] = SHF_R
    ct[:, _C_CZH:_C_CZH + 8] = 0.5 * np.cos(SHF_Z)
    ct[:, _C_SZH:_C_SZH + 8] = 0.5 * np.sin(SHF_Z)
    mask = np.zeros((128, 24), np.float32)
    for mb in range(4):
        for j in range(24):
            mask[mb * 24 + j, j] = 100.0
    ct[:, _C_MASK:_C_MASK + 24] = mask
    K = _POLY_K
    ct[:, _C_F2B] = np.log(2.0) + 4.0 * np.log(abs(K))
    ct[:, _C_RADB] = np.log(0.25) + 2.0 * np.log(abs(K))
    return ct


# ============================================================
# host-side indexing prep (no float math enters the output path)
# ============================================================

def _prep(species, coordinates):
    sp = np.asarray(species)
    co = np.asarray(coordinates, np.float32)
    cod = co.astype(np.float64)
    vec = cod[:, None, :, :] - cod[:, :, None, :]       # [m, i, j, 3] = r_j - r_i
    dmat = np.sqrt(np.maximum((vec ** 2).sum(-1), 0.0))
    adj = (dmat <= RCA) & ~np.eye(A, dtype=bool)[None]

    # per-(m, i) neighbor lists and per-half triple counts
    nbrs = [[np.where(adj[m, i])[0] for i in range(A)] for m in range(M)]
    tri_mi = np.array([[len(nbrs[m][i]) * (len(nbrs[m][i]) - 1) // 2
                        for i in range(A)] for m in range(M)], np.int64)
    Th = np.stack([tri_mi[:, :12].sum(1), tri_mi[:, 12:].sum(1)], 1)  # [M, 2]

    # molecule -> (core, slot): sort by total triples, deal rank-groups of 8
    order = np.argsort(-(Th.sum(1)), kind="stable")
    slot2mol = np.empty((NCORES, MLOC), np.int64)
    for s in range(MLOC):
        for c in range(NCORES):
            slot2mol[c, s] = order[s * NCORES + c]

    # chunks per group (uniform across cores)
    cpg = np.empty(NG, np.int64)
    for s in range(MLOC):
        for h in range(2):
            t = Th[slot2mol[:, s], h]
            cpg[2 * s + h] = max(1, int(np.ceil(t.max() / 128.0)))
    nch = int(cpg.sum())

    # flat triple arrays per core
    pj = np.zeros((NCORES, 128, nch, 3), np.float32)
    pk = np.zeros((NCORES, 128, nch, 3), np.float32)
    ci = np.zeros((NCORES, 128, nch, 3), np.float32)
    oh = np.zeros((NCORES, 128, nch, GSEG), ml_dtypes.bfloat16)

    gstart = np.concatenate([[0], np.cumsum(cpg)])
    for c in range(NCORES):
        for s in range(MLOC):
            m = slot2mol[c, s]
            for h in range(2):
                g = 2 * s + h
                base = gstart[g] * 128
                pos = 0
                for u in range(12):
                    i = h * 12 + u
                    nb = nbrs[m][i]
                    if len(nb) < 2:
                        continue
                    jj, kk = np.triu_indices(len(nb), 1)
                    j, k = nb[jj], nb[kk]
                    p = _TRIU[sp[m, j], sp[m, k]]
                    o = np.argsort(p, kind="stable")
                    j, k, p = j[o], k[o], p[o]
                    n = len(j)
                    sl = slice(base + pos, base + pos + n)
                    t_idx = np.arange(base + pos, base + pos + n)
                    chs, ts = t_idx // 128, t_idx % 128
                    pj[c, ts, chs] = co[m, j]
                    pk[c, ts, chs] = co[m, k]
                    ci[c, ts, chs] = np.broadcast_to(co[m, i], (n, 3))
                    oh[c, ts, chs, p * 12 + u] = 1
                    pos += n
                # pad remainder of the group: far-away fake pair -> w == 0,
                # one-hot row all-zero -> contributes nothing anyway
                tot = cpg[g] * 128
                if pos < tot:
                    t_idx = np.arange(base + pos, base + tot)
                    chs, ts = t_idx // 128, t_idx % 128
                    ref = co[m, 0]
                    pj[c, ts, chs] = ref + np.array([50, 0, 0], np.float32)
                    pk[c, ts, chs] = ref + np.array([0, 50, 0], np.float32)
                    ci[c, ts, chs] = ref

    # ---- radial inputs ----
    # rows: (molecule-in-block mb 0..3, atom j 0..23); groups of 4 slots
    rcj = np.zeros((NCORES, RGRP, 96, 3), np.float32)    # coords of atom j
    rcb = np.zeros((NCORES, RGRP, 96, 72), np.float32)   # molecule coords, (c,i)
    rsp = np.zeros((NCORES, RGRP, 96, 16), ml_dtypes.bfloat16)  # block-diag onehot
    for c in range(NCORES):
        for g in range(RGRP):
            for mb in range(4):
                m = slot2mol[c, g * 4 + mb]
                rows = slice(mb * 24, mb * 24 + 24)
                rcj[c, g, rows] = co[m]
                rcb[c, g, rows] = np.broadcast_to(
                    co[m].T.reshape(-1), (24, 72))
                rsp[c, g, np.arange(mb * 24, mb * 24 + 24),
                    mb * 4 + sp[m]] = 1

    meta = dict(nch=nch, cpg=tuple(int(x) for x in cpg), slot2mol=slot2mol)
    arrays = dict(pj=pj, pk=pk, ci=ci, oh=oh, rcj=rcj, rcb=rcb, rsp=rsp)
    return meta, arrays


# ============================================================
# device program
# ============================================================

def _bb(ap, dims, off=0):
    """Build a broadcast/strided view: keep ap's partition dim, replace free
    dims with explicit [step, count] pairs (element units)."""
    return AP(ap.tensor, ap.offset + off,
              [list(ap.ap[0])] + [list(d) for d in dims])


def _build(nch, cpg):
    nc = bacc.Bacc(None, target_bir_lowering=False)
    pj_d = nc.declare_dram_parameter("pj", [128, nch, 3], dt.float32, False)
    pk_d = nc.declare_dram_parameter("pk", [128, nch, 3], dt.float32, False)
    ci_d = nc.declare_dram_parameter("ci", [128, nch, 3], dt.float32, False)
    oh_d = nc.declare_dram_parameter("oh", [128, nch, GSEG], dt.bfloat16, False)
    rcj_d = nc.declare_dram_parameter("rcj", [RGRP, 96, 3], dt.float32, False)
    rcb_d = nc.declare_dram_parameter("rcb", [RGRP, 96, 72], dt.float32, False)
    rsp_d = nc.declare_dram_parameter("rsp", [RGRP, 96, 16], dt.bfloat16, False)
    ct_d = nc.declare_dram_parameter("consts", [128, _C_W], dt.float32, False)
    outa_d = nc.declare_dram_parameter("outa", [GSEG, NG * 32], dt.float32,
                                       True)
    outr_d = nc.declare_dram_parameter("outr", [16, RGRP * 384], dt.float32,
                                       True)

    gstart = [0]
    for g in range(NG):
        gstart.append(gstart[-1] + cpg[g])

    # block partition of the 64 groups
    gpb = NG // NBLK
    K, a = _POLY_K, _POLY_A
    # fold 2*K^4 (w = 2*fc_j*fc_k = 2*(K^2 s4j^2)(K^2 s4k^2)) into f2's exp bias
    F2BIAS = float(np.log(2.0) + 4.0 * np.log(abs(K)))
    # radial: rad = 0.25 * fc * exp(...) ; fc = (K*s4)^2
    RADBIAS = float(np.log(0.25) + 2.0 * np.log(abs(K)))

    with tile.TileContext(nc) as tc:
        with (
            tc.tile_pool(name="const", bufs=1) as cpool,
            tc.tile_pool(name="io", bufs=1) as io,
            tc.tile_pool(name="geo", bufs=1) as geo,
            tc.tile_pool(name="feat", bufs=1) as feat,
            tc.tile_pool(name="stg", bufs=1) as stg,
            tc.tile_pool(name="ps", bufs=4, space="PSUM") as ps,
        ):
            CT = cpool.tile([128, _C_W], dt.float32)
            nc.sync.dma_start(CT[:], ct_d[:])

            AZSTG = stg.tile([GSEG, NG * 32], dt.float32)   # angular staging
            RDSTG = stg.tile([16, RGRP * 384], dt.float32)  # radial staging

            V = nc.vector
            S = nc.scalar

            # angular inputs first: the geometry chain is the critical path
            PJL = io.tile([128, 3 * nch], dt.float32, tag="pj")
            PKL = io.tile([128, 3 * nch], dt.float32, tag="pk")
            CIL = io.tile([128, 3 * nch], dt.float32, tag="ci")
            OHL = io.tile([128, GSEG * nch], dt.bfloat16, tag="oh")
            nc.sync.dma_start(
                PJL[:].rearrange("p (n c) -> p n c", c=3), pj_d[:])
            nc.sync.dma_start(
                CIL[:].rearrange("p (n c) -> p n c", c=3), ci_d[:])
            nc.sync.dma_start(
                PKL[:].rearrange("p (n c) -> p n c", c=3), pk_d[:])
            nc.sync.dma_start(
                OHL[:].rearrange("p (n s) -> p n s", s=GSEG), oh_d[:])

            def poly_fc(dist, nb, tag, rc, npart=128):
                """fc up to factor K^2: returns s4 with fc = (K*s4)^2."""
                u = geo.tile([npart, nb], dt.float32, tag=tag + "_u")
                # (d min rc) mult (1/rc)
                V.tensor_scalar(u[:], dist[:], rc, 1.0 / rc, ALU.min,
                                ALU.mult)
                v = geo.tile([npart, nb], dt.float32, tag=tag + "_v")
                V.tensor_tensor(v[:], u[:], u[:], ALU.mult)
                acc = geo.tile([npart, nb], dt.float32, tag=tag + "_acc")
                V.scalar_tensor_tensor(acc[:], v[:], float(a[3]), v[:],
                                       ALU.add, ALU.mult)
                V.scalar_tensor_tensor(acc[:], acc[:], float(a[2]), v[:],
                                       ALU.add, ALU.mult)
                V.scalar_tensor_tensor(acc[:], acc[:], float(a[1]), v[:],
                                       ALU.add, ALU.mult)
                V.tensor_scalar(acc[:], acc[:], float(a[0]), None, ALU.add)
                return acc

            # ---------------- radial (all 8 groups batched) ----------------
            f32, bf16 = dt.float32, dt.bfloat16
            CJ = io.tile([96, RGRP * 3], f32, tag="rcj")
            CB = io.tile([96, RGRP * 72], f32, tag="rcb")
            SP = io.tile([96, RGRP * 16], bf16, tag="rsp")
            nc.sync.dma_start(CJ[:].rearrange("p (g x) -> p g x", x=3),
                              rcj_d[:].rearrange("g p x -> p g x"))
            nc.sync.dma_start(CB[:].rearrange("p (g x) -> p g x", x=72),
                              rcb_d[:].rearrange("g p x -> p g x"))
            nc.sync.dma_start(SP[:].rearrange("p (g x) -> p g x", x=16),
                              rsp_d[:].rearrange("g p x -> p g x"))

            rv = geo.tile([96, RGRP * 72], f32, tag="rv")
            V.tensor_tensor(
                _bb(rv[:], [[72, RGRP], [24, 3], [1, 24]]),
                _bb(CJ[:], [[3, RGRP], [1, 3], [0, 24]]),
                _bb(CB[:], [[72, RGRP], [24, 3], [1, 24]]),
                ALU.subtract)
            V.tensor_tensor(rv[:], rv[:], rv[:], ALU.mult)
            rd2 = geo.tile([96, RGRP * 24], f32, tag="rd2")
            V.tensor_tensor(rd2[:],
                            _bb(rv[:], [[72, RGRP], [1, 24]], off=0),
                            _bb(rv[:], [[72, RGRP], [1, 24]], off=24),
                            ALU.add)
            V.tensor_tensor(rd2[:], rd2[:],
                            _bb(rv[:], [[72, RGRP], [1, 24]], off=48),
                            ALU.add)
            V.tensor_tensor(rd2[:], rd2[:],
                            _bb(CT[:96, _C_MASK:], [[0, RGRP], [1, 24]]),
                            ALU.add)
            rdist = geo.tile([96, RGRP * 24], f32, tag="rdist")
            S.activation(rdist[:], rd2[:], AF.Ln)
            S.activation(rdist[:], rdist[:], AF.Exp, scale=0.5)

            rs4 = poly_fc(rdist, RGRP * 24, "rf", RCR, npart=96)
            rfc = geo.tile([96, RGRP * 24], f32, tag="rfc")
            V.tensor_tensor(rfc[:], rs4[:], rs4[:], ALU.mult)

            rt = feat.tile([96, RGRP * 384], f32, tag="rt")
            V.tensor_tensor(
                _bb(rt[:], [[384, RGRP], [16, 24], [1, 16]]),
                _bb(rdist[:], [[24, RGRP], [1, 24], [0, 16]]),
                _bb(CT[:96, _C_SHFR:], [[0, RGRP], [0, 24], [1, 16]]),
                ALU.subtract)
            rt2 = feat.tile([96, RGRP * 384], f32, tag="rt2")
            S.activation(rt2[:], rt[:], AF.Square)
            rex = feat.tile([96, RGRP * 384], f32, tag="rex")
            S.activation(rex[:], rt2[:], AF.Exp, scale=-ETA_R,
                         bias=CT[:96, _C_RADB:_C_RADB + 1])
            rad = feat.tile([96, RGRP * 384], bf16, tag="rad")
            V.tensor_tensor(
                _bb(rad[:], [[384, RGRP], [16, 24], [1, 16]]),
                _bb(rfc[:], [[24, RGRP], [1, 24], [0, 16]]),
                _bb(rex[:], [[384, RGRP], [16, 24], [1, 16]]),
                ALU.mult)

            for g in range(RGRP):
                rpt = ps.tile([16, 384], dt.float32, tag="rps")
                nc.tensor.matmul(rpt[:], SP[:, 16 * g:16 * (g + 1)],
                                 rad[:, 384 * g:384 * (g + 1)],
                                 start=True, stop=True)
                sl = slice(384 * g, 384 * (g + 1))
                S.activation(RDSTG[:, sl], rpt[:], AF.Copy)
                nc.sync.dma_start(outr_d[:, sl], RDSTG[:, sl])



            for b in range(NBLK):
                g0, g1 = b * gpb, (b + 1) * gpb
                c0, c1 = gstart[g0], gstart[g1]
                nb = c1 - c0
                f32, bf16 = dt.float32, dt.bfloat16

                PJ, PK, CI, OH = PJL, PKL, CIL, OHL

                vj = geo.tile([128, 3 * nb], f32, tag="vj")
                vk = geo.tile([128, 3 * nb], f32, tag="vk")
                V.tensor_tensor(vj[:], PJ[:], CI[:], ALU.subtract)
                V.tensor_tensor(vk[:], PK[:], CI[:], ALU.subtract)

                sq = geo.tile([128, 3 * nb], f32, tag="sq")
                d2j = geo.tile([128, nb], f32, tag="d2j")
                d2k = geo.tile([128, nb], f32, tag="d2k")
                dot = geo.tile([128, nb], f32, tag="dot")
                def cplane(t, cc):
                    return _bb(t[:], [[3, nb]], off=cc)
                V.tensor_tensor(sq[:], vj[:], vj[:], ALU.mult)
                V.tensor_tensor(d2j[:], cplane(sq, 0), cplane(sq, 1), ALU.add)
                V.tensor_tensor(d2j[:], d2j[:], cplane(sq, 2), ALU.add)
                V.tensor_tensor(sq[:], vk[:], vk[:], ALU.mult)
                V.tensor_tensor(d2k[:], cplane(sq, 0), cplane(sq, 1), ALU.add)
                V.tensor_tensor(d2k[:], d2k[:], cplane(sq, 2), ALU.add)
                V.tensor_tensor(sq[:], vj[:], vk[:], ALU.mult)
                V.tensor_tensor(dot[:], cplane(sq, 0), cplane(sq, 1), ALU.add)
                V.tensor_tensor(dot[:], dot[:], cplane(sq, 2), ALU.add)

                # d, 1/d via single ln + two exps (one ACT table set total)
                lj = geo.tile([128, nb], f32, tag="lj")
                lk = geo.tile([128, nb], f32, tag="lk")
                dj = geo.tile([128, nb], f32, tag="dj")
                dk = geo.tile([128, nb], f32, tag="dk")
                rj = geo.tile([128, nb], f32, tag="rj")
                rk = geo.tile([128, nb], f32, tag="rk")
                S.activation(lj[:], d2j[:], AF.Ln)
                S.activation(lk[:], d2k[:], AF.Ln)
                S.activation(dj[:], lj[:], AF.Exp, scale=0.5)
                S.activation(dk[:], lk[:], AF.Exp, scale=0.5)
                S.activation(rj[:], lj[:], AF.Exp, scale=-0.5)
                S.activation(rk[:], lk[:], AF.Exp, scale=-0.5)

                cos = geo.tile([128, nb], f32, tag="cos")
                V.scalar_tensor_tensor(cos[:], dot[:], 0.95, rj[:],
                                       ALU.mult, ALU.mult)
                V.tensor_tensor(cos[:], cos[:], rk[:], ALU.mult)
                s2 = geo.tile([128, nb], f32, tag="s2")
                V.tensor_tensor(s2[:], cos[:], cos[:], ALU.mult)
                V.tensor_scalar(s2[:], s2[:], -1.0, 1.0, ALU.mult, ALU.add)
                sin = geo.tile([128, nb], f32, tag="sin")
                S.activation(sin[:], s2[:], AF.Ln)
                S.activation(sin[:], sin[:], AF.Exp, scale=0.5)

                s4j = poly_fc(dj, nb, "fj", RCA)
                s4k = poly_fc(dk, nb, "fk", RCA)
                w2 = geo.tile([128, nb], f32, tag="w2")
                V.tensor_tensor(w2[:], s4j[:], s4k[:], ALU.mult)
                V.tensor_tensor(w2[:], w2[:], w2[:], ALU.mult)

                usum = geo.tile([128, nb], f32, tag="usum")
                V.tensor_tensor(usum[:], dj[:], dk[:], ALU.add)

                # f2[a] = exp(-eta/4*(u - 2shf_a)^2 + F2BIAS), layout (n, a)
                t4 = feat.tile([128, 4 * nb], f32, tag="t4")
                V.tensor_tensor(
                    _bb(t4[:], [[4, nb], [1, 4]]),
                    _bb(usum[:], [[1, nb], [0, 4]]),
                    _bb(CT[:, _C_SHF2A:], [[0, nb], [1, 4]]),
                    ALU.subtract)
                V.tensor_tensor(t4[:], t4[:], t4[:], ALU.mult)
                f2 = feat.tile([128, 4 * nb], f32, tag="f2")
                S.activation(f2[:], t4[:], AF.Exp, scale=-ETA_A / 4.0,
                             bias=CT[:, _C_F2B:_C_F2B + 1])
                wf2 = feat.tile([128, 4 * nb], f32, tag="wf2")
                V.tensor_tensor(
                    _bb(wf2[:], [[4, nb], [1, 4]]),
                    _bb(w2[:], [[1, nb], [0, 4]]),
                    _bb(f2[:], [[4, nb], [1, 4]]),
                    ALU.mult)

                # f1[z] = ((1 + cos(theta - shf_z))/2)^zeta, layout (n, z)
                q8 = feat.tile([128, 8 * nb], f32, tag="q8")
                t8 = feat.tile([128, 8 * nb], f32, tag="t8")
                V.tensor_tensor(
                    _bb(t8[:], [[8, nb], [1, 8]]),
                    _bb(cos[:], [[1, nb], [0, 8]]),
                    _bb(CT[:, _C_CZH:], [[0, nb], [1, 8]]),
                    ALU.mult)
                V.tensor_tensor(
                    _bb(q8[:], [[8, nb], [1, 8]]),
                    _bb(sin[:], [[1, nb], [0, 8]]),
                    _bb(CT[:, _C_SZH:], [[0, nb], [1, 8]]),
                    ALU.mult)
                V.scalar_tensor_tensor(q8[:], t8[:], 0.5, q8[:],
                                       ALU.add, ALU.add)
                S.activation(q8[:], q8[:], AF.Ln)
                f1 = feat.tile([128, 8 * nb], f32, tag="f1")
                S.activation(f1[:], q8[:], AF.Exp, scale=float(ZETA))

                # G[n, a, z] = wf2[n, a] * f1[n, z]   (bf16), emitted in
                # per-psum-tile slices so PE/copies/DMA trail the DVE
                G = feat.tile([128, 32 * nb], bf16, tag="G")
                for gt in range(g0, g1, PGRP):
                    ca, cb = gstart[gt] - c0, gstart[min(gt + PGRP, g1)] - c0
                    V.tensor_tensor(
                        _bb(G[:, 32 * ca:], [[32, cb - ca], [8, 4], [1, 8]]),
                        _bb(wf2[:, 4 * ca:], [[4, cb - ca], [1, 4], [0, 8]]),
                        _bb(f1[:, 8 * ca:], [[8, cb - ca], [0, 4], [1, 8]]),
                        ALU.mult)
                    pt = ps.tile([GSEG, 32 * PGRP], dt.float32, tag="ps")
                    for g in range(gt, min(gt + PGRP, g1)):
                        gi = g - gt
                        for k in range(cpg[g]):
                            cl = gstart[g] - c0 + k
                            nc.tensor.matmul(
                                pt[:, 32 * gi:32 * (gi + 1)],
                                OH[:, GSEG * cl:GSEG * (cl + 1)],
                                G[:, 32 * cl:32 * (cl + 1)],
                                start=(k == 0), stop=(k == cpg[g] - 1))
                    gbase = gt
                    sl = slice(32 * gbase, 32 * (gbase + PGRP))
                    S.activation(AZSTG[:, sl], pt[:], AF.Copy)
                    nc.sync.dma_start(outa_d[:, sl], AZSTG[:, sl])

    _patch_act_tables()
    nc.compile()
    return nc


_ACT_PATCHED = False


def _patch_act_tables():
    """Make Ln/Exp resolve only to the combined natural_log_exp set, so the
    table-load pass emits ONE load instead of thrashing between the ln-only
    and exp-only sets (1.28us per reload)."""
    global _ACT_PATCHED
    if _ACT_PATCHED:
        return
    orig = bacc.get_activation_tables

    def patched(arch):
        t = dict(orig(arch))
        out = {}
        for name, fns in t.items():
            if name != "natural_log_exp_and_others":
                fns = {f for f in fns if f not in (AF.Ln, AF.Exp)}
            out[name] = fns
        return out

    bacc.get_activation_tables = patched
    _ACT_PATCHED = True


_CACHE = {}


def kernel(species, coordinates, coefficients=None):
    species = np.asarray(species)
    coordinates = np.asarray(coordinates, np.float32)
    meta, arrays = _prep(species, coordinates)
    key = (meta["nch"], meta["cpg"])
    if key not in _CACHE:
        _CACHE[key] = _build(meta["nch"], list(meta["cpg"]))
    nc = _CACHE[key]

    ct = _build_consts()
    in_maps = []
    for c in range(NCORES):
        in_maps.append({
            "pj": arrays["pj"][c], "pk": arrays["pk"][c],
            "ci": arrays["ci"][c], "oh": arrays["oh"][c],
            "rcj": arrays["rcj"][c], "rcb": arrays["rcb"][c],
            "rsp": arrays["rsp"][c], "consts": ct,
        })
    res = run_bass_kernel_spmd(nc, in_maps, core_ids=list(range(NCORES)))
    out = np.empty((M, A, 384), np.float32)
    for c in range(NCORES):
        outa = np.asarray(res.results[c]["outa"])  # [128, NG*32]
        outr = np.asarray(res.results[c]["outr"])  # [16, RGRP*384]
        ang = outa.reshape(GSEG, NG, 32)[:120]
        ang = ang.reshape(10, 12, MLOC, 2, 32)          # [p, u, s, h, az]
        ang = ang.transpose(2, 3, 1, 0, 4).reshape(MLOC, A, 320)
        rad = outr.reshape(4, 4, RGRP, A, 16)           # [mb, sp, g, i, r]
        rad = rad.transpose(2, 0, 3, 1, 4).reshape(MLOC, A, 64)
        out[meta["slot2mol"][c], :, :64] = rad
        out[meta["slot2mol"][c], :, 64:] = ang
    return out


# revision 38
# speedup vs baseline: 1.1317x; 1.1317x over previous
"""ANI-style AEV computer (radial + angular) on 8 Trainium2 NeuronCores.

Strategy
--------
Data-parallel over molecules (32/core), with host-side *indexing only*
(neighborlists / triple lists / one-hot bin matrices); every floating-point
operation of the AEV math runs on-device.

Angular part: the all-triples tensor is ~94% zeros under the Rca=3.5 cutoff,
so the host enumerates surviving triples (center i, neighbors j<k) into a
flat per-core list, sorted by (molecule-slot, center-half, center, species
-pair-bin).  The device computes, per 128-triple chunk:
  geometry (vectors, d^2, dot) -> 1/d and d via ACT ln/exp -> cos/sin of the
  angle -> cutoff poly -> f2 = exp(-eta (davg-shf)^2) -> f1 = q^zeta via
  exp(zeta*ln q) -> G = w*f2 (x) f1  [bf16, 32 features]
and bins G into (center, species-pair) segments with a PE matmul against a
one-hot segment matrix (PSUM-accumulated across a segment-group's chunks).

Radial part: dense over all (i,j) pairs, species-binned with a small
block-diagonal one-hot matmul.

Only one ACT table set is used (natural_log_exp): cutoff cosines are
evaluated as a degree-4 Chebyshev polynomial in u^2 (error ~1e-6), which
keeps the Activation engine free of table switches.
"""

import os
import sys

import numpy as np

for _p in ("/opt/trn_rl_repo", "/root/.axon_site/_ro/trn_rl_repo"):
    if os.path.isdir(_p) and _p not in sys.path:
        sys.path.insert(0, _p)

import concourse.bass as bass
import concourse.mybir as mybir
from concourse import bacc, tile
from concourse.bass_utils import run_bass_kernel_spmd

import ml_dtypes

AF = mybir.ActivationFunctionType
ALU = mybir.AluOpType
dt = mybir.dt
AP = bass.AP

# ---- hyperparameters (match reference) ----
NCORES = 8
M, A = 256, 24
MLOC = M // NCORES          # 32 molecules per core
RCR, RCA = 5.2, 3.5
ETA_R, ETA_A, ZETA = 16.0, 8.0, 32.0
SHF_R = np.linspace(0.9, 5.2, 17)[:-1].astype(np.float64)   # 16
SHF_A = np.linspace(0.9, 3.5, 5)[:-1].astype(np.float64)    # 4
SHF_Z = (np.arange(8) + 0.5) * np.pi / 8.0                   # 8
NPAIR, RSUB, ASUB = 10, 16, 32
NSEG = 120                  # segments per psum group = 12 centers x 10 bins
GSEG = 128                  # one-hot width (8 pad cols -> FWL weight loads)
NG = 2 * MLOC               # 64 groups/core (2 per molecule slot)
NBLK = 1                    # angular emission blocks
PGRP = 16                   # psum groups packed per PSUM bank tile
RGRP = MLOC // 4            # 8 radial groups of 4 molecules (96 = 4*24 rows)

_TRIU = np.zeros((4, 4), np.int64)
_s1, _s2 = np.triu_indices(4)
_TRIU[_s1, _s2] = np.arange(len(_s1))
_TRIU[_s2, _s1] = _TRIU[_s1, _s2]

# ---- degree-4 (in v=u^2) Chebyshev fit of cos(pi*u/2) on u in [0,1] ----
def _cos_poly():
    v = np.linspace(0.0, 1.0, 4001)
    tgt = np.cos(0.5 * np.pi * np.sqrt(v))
    from numpy.polynomial import chebyshev as C
    ch = C.Chebyshev.fit(v, tgt, 4, domain=[0, 1])
    pw = ch.convert(kind=np.polynomial.Polynomial)
    c = pw.coef  # c0..c4 in v
    K = c[4]
    a = c[:4] / K  # monic residual coeffs a0..a3
    err = np.abs(np.polyval(c[::-1], v) - tgt).max()
    return K, a, err

_POLY_K, _POLY_A, _POLY_ERR = _cos_poly()

# const tile column map ([128, 60] fp32)
_C_SHF2A = 0     # 4  : 2*shf_a
_C_SHFR = 4      # 16 : shf_r
_C_CZH = 20      # 8  : 0.5*cos(shf_z)
_C_SZH = 28      # 8  : 0.5*sin(shf_z)
_C_MASK = 36     # 24 : radial i==j mask*100 (valid on partitions 0..95)
_C_F2B = 60      # 1  : angular exp bias ln(2*K^4)
_C_RADB = 61     # 1  : radial exp bias ln(0.25*K^2)
_C_W = 62


def _build_consts():
    ct = np.zeros((128, _C_W), np.float32)
    ct[:, _C_SHF2A:_C_SHF2A + 4] = 2.0 * SHF_A
    ct[:, _C_SHFR:_C_SHFR + 16] = SHF_R
    ct[:, _C_CZH:_C_CZH + 8] = 0.5 * np.cos(SHF_Z)
    ct[:, _C_SZH:_C_SZH + 8] = 0.5 * np.sin(SHF_Z)
    mask = np.zeros((128, 24), np.float32)
    for mb in range(4):
        for j in range(24):
            mask[mb * 24 + j, j] = 100.0
    ct[:, _C_MASK:_C_MASK + 24] = mask
    K = _POLY_K
    ct[:, _C_F2B] = np.log(2.0) + 4.0 * np.log(abs(K))
    ct[:, _C_RADB] = np.log(0.25) + 2.0 * np.log(abs(K))
    return ct


# ============================================================
# host-side indexing prep (no float math enters the output path)
# ============================================================

def _prep(species, coordinates):
    sp = np.asarray(species)
    co = np.asarray(coordinates, np.float32)
    cod = co.astype(np.float64)
    vec = cod[:, None, :, :] - cod[:, :, None, :]       # [m, i, j, 3] = r_j - r_i
    dmat = np.sqrt(np.maximum((vec ** 2).sum(-1), 0.0))
    adj = (dmat <= RCA) & ~np.eye(A, dtype=bool)[None]

    # per-(m, i) neighbor lists and per-half triple counts
    nbrs = [[np.where(adj[m, i])[0] for i in range(A)] for m in range(M)]
    tri_mi = np.array([[len(nbrs[m][i]) * (len(nbrs[m][i]) - 1) // 2
                        for i in range(A)] for m in range(M)], np.int64)
    Th = np.stack([tri_mi[:, :12].sum(1), tri_mi[:, 12:].sum(1)], 1)  # [M, 2]

    # molecule -> (core, slot): sort by total triples, deal rank-groups of 8
    order = np.argsort(-(Th.sum(1)), kind="stable")
    slot2mol = np.empty((NCORES, MLOC), np.int64)
    for s in range(MLOC):
        for c in range(NCORES):
            slot2mol[c, s] = order[s * NCORES + c]

    # chunks per group (uniform across cores)
    cpg = np.empty(NG, np.int64)
    for s in range(MLOC):
        for h in range(2):
            t = Th[slot2mol[:, s], h]
            cpg[2 * s + h] = max(1, int(np.ceil(t.max() / 128.0)))
    nch = int(cpg.sum())

    # flat triple arrays per core
    pj = np.zeros((NCORES, 128, nch, 3), np.float32)
    pk = np.zeros((NCORES, 128, nch, 3), np.float32)
    ci = np.zeros((NCORES, 128, nch, 3), np.float32)
    oh = np.zeros((NCORES, 128, nch, GSEG), ml_dtypes.bfloat16)

    gstart = np.concatenate([[0], np.cumsum(cpg)])
    for c in range(NCORES):
        for s in range(MLOC):
            m = slot2mol[c, s]
            for h in range(2):
                g = 2 * s + h
                base = gstart[g] * 128
                pos = 0
                for u in range(12):
                    i = h * 12 + u
                    nb = nbrs[m][i]
                    if len(nb) < 2:
                        continue
                    jj, kk = np.triu_indices(len(nb), 1)
                    j, k = nb[jj], nb[kk]
                    p = _TRIU[sp[m, j], sp[m, k]]
                    o = np.argsort(p, kind="stable")
                    j, k, p = j[o], k[o], p[o]
                    n = len(j)
                    sl = slice(base + pos, base + pos + n)
                    t_idx = np.arange(base + pos, base + pos + n)
                    chs, ts = t_idx // 128, t_idx % 128
                    pj[c, ts, chs] = co[m, j]
                    pk[c, ts, chs] = co[m, k]
                    ci[c, ts, chs] = np.broadcast_to(co[m, i], (n, 3))
                    oh[c, ts, chs, p * 12 + u] = 1
                    pos += n
                # pad remainder of the group: far-away fake pair -> w == 0,
                # one-hot row all-zero -> contributes nothing anyway
                tot = cpg[g] * 128
                if pos < tot:
                    t_idx = np.arange(base + pos, base + tot)
                    chs, ts = t_idx // 128, t_idx % 128
                    ref = co[m, 0]
                    pj[c, ts, chs] = ref + np.array([50, 0, 0], np.float32)
                    pk[c, ts, chs] = ref + np.array([0, 50, 0], np.float32)
                    ci[c, ts, chs] = ref

    # ---- radial inputs ----
    # rows: (molecule-in-block mb 0..3, atom j 0..23); groups of 4 slots
    rcj = np.zeros((NCORES, RGRP, 96, 3), np.float32)    # coords of atom j
    rcb = np.zeros((NCORES, RGRP, 96, 72), np.float32)   # molecule coords, (c,i)
    rsp = np.zeros((NCORES, RGRP, 96, 16), ml_dtypes.bfloat16)  # block-diag onehot
    for c in range(NCORES):
        for g in range(RGRP):
            for mb in range(4):
                m = slot2mol[c, g * 4 + mb]
                rows = slice(mb * 24, mb * 24 + 24)
                rcj[c, g, rows] = co[m]
                rcb[c, g, rows] = np.broadcast_to(
                    co[m].T.reshape(-1), (24, 72))
                rsp[c, g, np.arange(mb * 24, mb * 24 + 24),
                    mb * 4 + sp[m]] = 1

    meta = dict(nch=nch, cpg=tuple(int(x) for x in cpg), slot2mol=slot2mol)
    arrays = dict(pj=pj, pk=pk, ci=ci, oh=oh, rcj=rcj, rcb=rcb, rsp=rsp)
    return meta, arrays


# ============================================================
# device program
# ============================================================

def _bb(ap, dims, off=0):
    """Build a broadcast/strided view: keep ap's partition dim, replace free
    dims with explicit [step, count] pairs (element units)."""
    return AP(ap.tensor, ap.offset + off,
              [list(ap.ap[0])] + [list(d) for d in dims])


def _build(nch, cpg):
    nc = bacc.Bacc(None, target_bir_lowering=False)
    pj_d = nc.declare_dram_parameter("pj", [128, nch, 3], dt.float32, False)
    pk_d = nc.declare_dram_parameter("pk", [128, nch, 3], dt.float32, False)
    ci_d = nc.declare_dram_parameter("ci", [128, nch, 3], dt.float32, False)
    oh_d = nc.declare_dram_parameter("oh", [128, nch, GSEG], dt.bfloat16, False)
    rcj_d = nc.declare_dram_parameter("rcj", [RGRP, 96, 3], dt.float32, False)
    rcb_d = nc.declare_dram_parameter("rcb", [RGRP, 96, 72], dt.float32, False)
    rsp_d = nc.declare_dram_parameter("rsp", [RGRP, 96, 16], dt.bfloat16, False)
    ct_d = nc.declare_dram_parameter("consts", [128, _C_W], dt.float32, False)
    outa_d = nc.declare_dram_parameter("outa", [GSEG, NG * 32], dt.float32,
                                       True)
    outr_d = nc.declare_dram_parameter("outr", [16, RGRP * 384], dt.float32,
                                       True)

    gstart = [0]
    for g in range(NG):
        gstart.append(gstart[-1] + cpg[g])

    # block partition of the 64 groups
    gpb = NG // NBLK
    K, a = _POLY_K, _POLY_A
    # fold 2*K^4 (w = 2*fc_j*fc_k = 2*(K^2 s4j^2)(K^2 s4k^2)) into f2's exp bias
    F2BIAS = float(np.log(2.0) + 4.0 * np.log(abs(K)))
    # radial: rad = 0.25 * fc * exp(...) ; fc = (K*s4)^2
    RADBIAS = float(np.log(0.25) + 2.0 * np.log(abs(K)))

    with tile.TileContext(nc) as tc:
        with (
            tc.tile_pool(name="const", bufs=1) as cpool,
            tc.tile_pool(name="io", bufs=1) as io,
            tc.tile_pool(name="geo", bufs=1) as geo,
            tc.tile_pool(name="feat", bufs=1) as feat,
            tc.tile_pool(name="stg", bufs=1) as stg,
            tc.tile_pool(name="ps", bufs=4, space="PSUM") as ps,
        ):
            CT = cpool.tile([128, _C_W], dt.float32)
            nc.sync.dma_start(CT[:], ct_d[:])

            AZSTG = stg.tile([GSEG, NG * 32], dt.float32)   # angular staging
            RDSTG = stg.tile([16, RGRP * 384], dt.float32)  # radial staging

            V = nc.vector
            S = nc.scalar

            # angular inputs first: the geometry chain is the critical path
            PJL = io.tile([128, 3 * nch], dt.float32, tag="pj")
            PKL = io.tile([128, 3 * nch], dt.float32, tag="pk")
            CIL = io.tile([128, 3 * nch], dt.float32, tag="ci")
            OHL = io.tile([128, GSEG * nch], dt.bfloat16, tag="oh")
            nc.sync.dma_start(
                PJL[:].rearrange("p (n c) -> p n c", c=3), pj_d[:])
            nc.sync.dma_start(
                CIL[:].rearrange("p (n c) -> p n c", c=3), ci_d[:])
            nc.sync.dma_start(
                PKL[:].rearrange("p (n c) -> p n c", c=3), pk_d[:])

            def poly_fc(dist, nb, tag, rc, npart=128):
                """fc up to factor K^2: returns s4 with fc = (K*s4)^2."""
                u = geo.tile([npart, nb], dt.float32, tag=tag + "_u")
                # (d min rc) mult (1/rc)
                V.tensor_scalar(u[:], dist[:], rc, 1.0 / rc, ALU.min,
                                ALU.mult)
                v = geo.tile([npart, nb], dt.float32, tag=tag + "_v")
                V.tensor_tensor(v[:], u[:], u[:], ALU.mult)
                acc = geo.tile([npart, nb], dt.float32, tag=tag + "_acc")
                V.scalar_tensor_tensor(acc[:], v[:], float(a[3]), v[:],
                                       ALU.add, ALU.mult)
                V.scalar_tensor_tensor(acc[:], acc[:], float(a[2]), v[:],
                                       ALU.add, ALU.mult)
                V.scalar_tensor_tensor(acc[:], acc[:], float(a[1]), v[:],
                                       ALU.add, ALU.mult)
                V.tensor_scalar(acc[:], acc[:], float(a[0]), None, ALU.add)
                return acc

            # ---------------- radial (all 8 groups batched) ----------------
            f32, bf16 = dt.float32, dt.bfloat16
            CJ = io.tile([96, RGRP * 3], f32, tag="rcj")
            CB = io.tile([96, RGRP * 72], f32, tag="rcb")
            SP = io.tile([96, RGRP * 16], bf16, tag="rsp")
            nc.sync.dma_start(CJ[:].rearrange("p (g x) -> p g x", x=3),
                              rcj_d[:].rearrange("g p x -> p g x"))
            nc.sync.dma_start(CB[:].rearrange("p (g x) -> p g x", x=72),
                              rcb_d[:].rearrange("g p x -> p g x"))
            nc.sync.dma_start(SP[:].rearrange("p (g x) -> p g x", x=16),
                              rsp_d[:].rearrange("g p x -> p g x"))
            nc.sync.dma_start(
                OHL[:].rearrange("p (n s) -> p n s", s=GSEG), oh_d[:])

            rv = geo.tile([96, RGRP * 72], f32, tag="rv")
            V.tensor_tensor(
                _bb(rv[:], [[72, RGRP], [24, 3], [1, 24]]),
                _bb(CJ[:], [[3, RGRP], [1, 3], [0, 24]]),
                _bb(CB[:], [[72, RGRP], [24, 3], [1, 24]]),
                ALU.subtract)
            V.tensor_tensor(rv[:], rv[:], rv[:], ALU.mult)
            rd2 = geo.tile([96, RGRP * 24], f32, tag="rd2")
            V.tensor_tensor(rd2[:],
                            _bb(rv[:], [[72, RGRP], [1, 24]], off=0),
                            _bb(rv[:], [[72, RGRP], [1, 24]], off=24),
                            ALU.add)
            V.tensor_tensor(rd2[:], rd2[:],
                            _bb(rv[:], [[72, RGRP], [1, 24]], off=48),
                            ALU.add)
            V.tensor_tensor(rd2[:], rd2[:],
                            _bb(CT[:96, _C_MASK:], [[0, RGRP], [1, 24]]),
                            ALU.add)
            rdist = geo.tile([96, RGRP * 24], f32, tag="rdist")
            S.activation(rdist[:], rd2[:], AF.Ln)
            S.activation(rdist[:], rdist[:], AF.Exp, scale=0.5)

            rs4 = poly_fc(rdist, RGRP * 24, "rf", RCR, npart=96)
            rfc = geo.tile([96, RGRP * 24], f32, tag="rfc")
            V.tensor_tensor(rfc[:], rs4[:], rs4[:], ALU.mult)

            rt = feat.tile([96, RGRP * 384], f32, tag="rt")
            V.tensor_tensor(
                _bb(rt[:], [[384, RGRP], [16, 24], [1, 16]]),
                _bb(rdist[:], [[24, RGRP], [1, 24], [0, 16]]),
                _bb(CT[:96, _C_SHFR:], [[0, RGRP], [0, 24], [1, 16]]),
                ALU.subtract)
            rt2 = feat.tile([96, RGRP * 384], f32, tag="rt2")
            S.activation(rt2[:], rt[:], AF.Square)
            rex = feat.tile([96, RGRP * 384], f32, tag="rex")
            S.activation(rex[:], rt2[:], AF.Exp, scale=-ETA_R,
                         bias=CT[:96, _C_RADB:_C_RADB + 1])
            rad = feat.tile([96, RGRP * 384], bf16, tag="rad")
            V.tensor_tensor(
                _bb(rad[:], [[384, RGRP], [16, 24], [1, 16]]),
                _bb(rfc[:], [[24, RGRP], [1, 24], [0, 16]]),
                _bb(rex[:], [[384, RGRP], [16, 24], [1, 16]]),
                ALU.mult)

            for g in range(RGRP):
                rpt = ps.tile([16, 384], dt.float32, tag="rps")
                nc.tensor.matmul(rpt[:], SP[:, 16 * g:16 * (g + 1)],
                                 rad[:, 384 * g:384 * (g + 1)],
                                 start=True, stop=True)
                sl = slice(384 * g, 384 * (g + 1))
                S.activation(RDSTG[:, sl], rpt[:], AF.Copy)
                nc.sync.dma_start(outr_d[:, sl], RDSTG[:, sl])



            for b in range(NBLK):
                g0, g1 = b * gpb, (b + 1) * gpb
                c0, c1 = gstart[g0], gstart[g1]
                nb = c1 - c0
                f32, bf16 = dt.float32, dt.bfloat16

                PJ, PK, CI, OH = PJL, PKL, CIL, OHL

                vj = geo.tile([128, 3 * nb], f32, tag="vj")
                vk = geo.tile([128, 3 * nb], f32, tag="vk")
                V.tensor_tensor(vj[:], PJ[:], CI[:], ALU.subtract)
                V.tensor_tensor(vk[:], PK[:], CI[:], ALU.subtract)

                sq = geo.tile([128, 3 * nb], f32, tag="sq")
                d2j = geo.tile([128, nb], f32, tag="d2j")
                d2k = geo.tile([128, nb], f32, tag="d2k")
                dot = geo.tile([128, nb], f32, tag="dot")
                def cplane(t, cc):
                    return _bb(t[:], [[3, nb]], off=cc)
                V.tensor_tensor(sq[:], vj[:], vj[:], ALU.mult)
                V.tensor_tensor(d2j[:], cplane(sq, 0), cplane(sq, 1), ALU.add)
                V.tensor_tensor(d2j[:], d2j[:], cplane(sq, 2), ALU.add)
                V.tensor_tensor(sq[:], vk[:], vk[:], ALU.mult)
                V.tensor_tensor(d2k[:], cplane(sq, 0), cplane(sq, 1), ALU.add)
                V.tensor_tensor(d2k[:], d2k[:], cplane(sq, 2), ALU.add)
                V.tensor_tensor(sq[:], vj[:], vk[:], ALU.mult)
                V.tensor_tensor(dot[:], cplane(sq, 0), cplane(sq, 1), ALU.add)
                V.tensor_tensor(dot[:], dot[:], cplane(sq, 2), ALU.add)

                # d, 1/d via single ln + two exps (one ACT table set total)
                lj = geo.tile([128, nb], f32, tag="lj")
                lk = geo.tile([128, nb], f32, tag="lk")
                dj = geo.tile([128, nb], f32, tag="dj")
                dk = geo.tile([128, nb], f32, tag="dk")
                rj = geo.tile([128, nb], f32, tag="rj")
                rk = geo.tile([128, nb], f32, tag="rk")
                S.activation(lj[:], d2j[:], AF.Ln)
                S.activation(lk[:], d2k[:], AF.Ln)
                S.activation(dj[:], lj[:], AF.Exp, scale=0.5)
                S.activation(dk[:], lk[:], AF.Exp, scale=0.5)
                S.activation(rj[:], lj[:], AF.Exp, scale=-0.5)
                S.activation(rk[:], lk[:], AF.Exp, scale=-0.5)

                cos = geo.tile([128, nb], f32, tag="cos")
                V.scalar_tensor_tensor(cos[:], dot[:], 0.95, rj[:],
                                       ALU.mult, ALU.mult)
                V.tensor_tensor(cos[:], cos[:], rk[:], ALU.mult)
                s2 = geo.tile([128, nb], f32, tag="s2")
                V.tensor_tensor(s2[:], cos[:], cos[:], ALU.mult)
                V.tensor_scalar(s2[:], s2[:], -1.0, 1.0, ALU.mult, ALU.add)
                sin = geo.tile([128, nb], f32, tag="sin")
                S.activation(sin[:], s2[:], AF.Ln)
                S.activation(sin[:], sin[:], AF.Exp, scale=0.5)

                s4j = poly_fc(dj, nb, "fj", RCA)
                s4k = poly_fc(dk, nb, "fk", RCA)
                w2 = geo.tile([128, nb], f32, tag="w2")
                V.tensor_tensor(w2[:], s4j[:], s4k[:], ALU.mult)
                V.tensor_tensor(w2[:], w2[:], w2[:], ALU.mult)

                usum = geo.tile([128, nb], f32, tag="usum")
                V.tensor_tensor(usum[:], dj[:], dk[:], ALU.add)

                # f2[a] = exp(-eta/4*(u - 2shf_a)^2 + F2BIAS), layout (n, a)
                t4 = feat.tile([128, 4 * nb], f32, tag="t4")
                V.tensor_tensor(
                    _bb(t4[:], [[4, nb], [1, 4]]),
                    _bb(usum[:], [[1, nb], [0, 4]]),
                    _bb(CT[:, _C_SHF2A:], [[0, nb], [1, 4]]),
                    ALU.subtract)
                V.tensor_tensor(t4[:], t4[:], t4[:], ALU.mult)
                f2 = feat.tile([128, 4 * nb], f32, tag="f2")
                S.activation(f2[:], t4[:], AF.Exp, scale=-ETA_A / 4.0,
                             bias=CT[:, _C_F2B:_C_F2B + 1])
                wf2 = feat.tile([128, 4 * nb], f32, tag="wf2")
                V.tensor_tensor(
                    _bb(wf2[:], [[4, nb], [1, 4]]),
                    _bb(w2[:], [[1, nb], [0, 4]]),
                    _bb(f2[:], [[4, nb], [1, 4]]),
                    ALU.mult)

                # f1[z] = ((1 + cos(theta - shf_z))/2)^zeta, layout (n, z)
                q8 = feat.tile([128, 8 * nb], f32, tag="q8")
                t8 = feat.tile([128, 8 * nb], f32, tag="t8")
                V.tensor_tensor(
                    _bb(t8[:], [[8, nb], [1, 8]]),
                    _bb(cos[:], [[1, nb], [0, 8]]),
                    _bb(CT[:, _C_CZH:], [[0, nb], [1, 8]]),
                    ALU.mult)
                V.tensor_tensor(
                    _bb(q8[:], [[8, nb], [1, 8]]),
                    _bb(sin[:], [[1, nb], [0, 8]]),
                    _bb(CT[:, _C_SZH:], [[0, nb], [1, 8]]),
                    ALU.mult)
                V.scalar_tensor_tensor(q8[:], t8[:], 0.5, q8[:],
                                       ALU.add, ALU.add)
                S.activation(q8[:], q8[:], AF.Ln)
                f1 = feat.tile([128, 8 * nb], f32, tag="f1")
                S.activation(f1[:], q8[:], AF.Exp, scale=float(ZETA))

                # G[n, a, z] = wf2[n, a] * f1[n, z]   (bf16), emitted in
                # per-psum-tile slices so PE/copies/DMA trail the DVE
                G = feat.tile([128, 32 * nb], bf16, tag="G")
                for gt in range(g0, g1, PGRP):
                    ca, cb = gstart[gt] - c0, gstart[min(gt + PGRP, g1)] - c0
                    V.tensor_tensor(
                        _bb(G[:, 32 * ca:], [[32, cb - ca], [8, 4], [1, 8]]),
                        _bb(wf2[:, 4 * ca:], [[4, cb - ca], [1, 4], [0, 8]]),
                        _bb(f1[:, 8 * ca:], [[8, cb - ca], [0, 4], [1, 8]]),
                        ALU.mult)
                    pt = ps.tile([GSEG, 32 * PGRP], dt.float32, tag="ps")
                    for g in range(gt, min(gt + PGRP, g1)):
                        gi = g - gt
                        for k in range(cpg[g]):
                            cl = gstart[g] - c0 + k
                            nc.tensor.matmul(
                                pt[:, 32 * gi:32 * (gi + 1)],
                                OH[:, GSEG * cl:GSEG * (cl + 1)],
                                G[:, 32 * cl:32 * (cl + 1)],
                                start=(k == 0), stop=(k == cpg[g] - 1))
                    gbase = gt
                    sl = slice(32 * gbase, 32 * (gbase + PGRP))
                    S.activation(AZSTG[:, sl], pt[:], AF.Copy)
                    nc.sync.dma_start(outa_d[:, sl], AZSTG[:, sl])

    _patch_act_tables()
    nc.compile()
    return nc


_ACT_PATCHED = False


def _patch_act_tables():
    """Make Ln/Exp resolve only to the combined natural_log_exp set, so the
    table-load pass emits ONE load instead of thrashing between the ln-only
    and exp-only sets (1.28us per reload)."""
    global _ACT_PATCHED
    if _ACT_PATCHED:
        return
    orig = bacc.get_activation_tables

    def patched(arch):
        t = dict(orig(arch))
        out = {}
        for name, fns in t.items():
            if name != "natural_log_exp_and_others":
                fns = {f for f in fns if f not in (AF.Ln, AF.Exp)}
            out[name] = fns
        return out

    bacc.get_activation_tables = patched
    _ACT_PATCHED = True


_CACHE = {}


def kernel(species, coordinates, coefficients=None):
    species = np.asarray(species)
    coordinates = np.asarray(coordinates, np.float32)
    meta, arrays = _prep(species, coordinates)
    key = (meta["nch"], meta["cpg"])
    if key not in _CACHE:
        _CACHE[key] = _build(meta["nch"], list(meta["cpg"]))
    nc = _CACHE[key]

    ct = _build_consts()
    in_maps = []
    for c in range(NCORES):
        in_maps.append({
            "pj": arrays["pj"][c], "pk": arrays["pk"][c],
            "ci": arrays["ci"][c], "oh": arrays["oh"][c],
            "rcj": arrays["rcj"][c], "rcb": arrays["rcb"][c],
            "rsp": arrays["rsp"][c], "consts": ct,
        })
    res = run_bass_kernel_spmd(nc, in_maps, core_ids=list(range(NCORES)))
    out = np.empty((M, A, 384), np.float32)
    for c in range(NCORES):
        outa = np.asarray(res.results[c]["outa"])  # [128, NG*32]
        outr = np.asarray(res.results[c]["outr"])  # [16, RGRP*384]
        ang = outa.reshape(GSEG, NG, 32)[:120]
        ang = ang.reshape(10, 12, MLOC, 2, 32)          # [p, u, s, h, az]
        ang = ang.transpose(2, 3, 1, 0, 4).reshape(MLOC, A, 320)
        rad = outr.reshape(4, 4, RGRP, A, 16)           # [mb, sp, g, i, r]
        rad = rad.transpose(2, 0, 3, 1, 4).reshape(MLOC, A, 64)
        out[meta["slot2mol"][c], :, :64] = rad
        out[meta["slot2mol"][c], :, 64:] = ang
    return out


# revision 39
# speedup vs baseline: 1.1615x; 1.0264x over previous
"""ANI-style AEV computer (radial + angular) on 8 Trainium2 NeuronCores.

Strategy
--------
Data-parallel over molecules (32/core), with host-side *indexing only*
(neighborlists / triple lists / one-hot bin matrices); every floating-point
operation of the AEV math runs on-device.

Angular part: the all-triples tensor is ~94% zeros under the Rca=3.5 cutoff,
so the host enumerates surviving triples (center i, neighbors j<k) into a
flat per-core list, sorted by (molecule-slot, center-half, center, species
-pair-bin).  The device computes, per 128-triple chunk:
  geometry (vectors, d^2, dot) -> 1/d and d via ACT ln/exp -> cos/sin of the
  angle -> cutoff poly -> f2 = exp(-eta (davg-shf)^2) -> f1 = q^zeta via
  exp(zeta*ln q) -> G = w*f2 (x) f1  [bf16, 32 features]
and bins G into (center, species-pair) segments with a PE matmul against a
one-hot segment matrix (PSUM-accumulated across a segment-group's chunks).

Radial part: dense over all (i,j) pairs, species-binned with a small
block-diagonal one-hot matmul.

Only one ACT table set is used (natural_log_exp): cutoff cosines are
evaluated as a degree-4 Chebyshev polynomial in u^2 (error ~1e-6), which
keeps the Activation engine free of table switches.
"""

import os
import sys

import numpy as np

for _p in ("/opt/trn_rl_repo", "/root/.axon_site/_ro/trn_rl_repo"):
    if os.path.isdir(_p) and _p not in sys.path:
        sys.path.insert(0, _p)

import concourse.bass as bass
import concourse.mybir as mybir
from concourse import bacc, tile
from concourse.bass_utils import run_bass_kernel_spmd

import ml_dtypes

AF = mybir.ActivationFunctionType
ALU = mybir.AluOpType
dt = mybir.dt
AP = bass.AP

# ---- hyperparameters (match reference) ----
NCORES = 8
M, A = 256, 24
MLOC = M // NCORES          # 32 molecules per core
RCR, RCA = 5.2, 3.5
ETA_R, ETA_A, ZETA = 16.0, 8.0, 32.0
SHF_R = np.linspace(0.9, 5.2, 17)[:-1].astype(np.float64)   # 16
SHF_A = np.linspace(0.9, 3.5, 5)[:-1].astype(np.float64)    # 4
SHF_Z = (np.arange(8) + 0.5) * np.pi / 8.0                   # 8
NPAIR, RSUB, ASUB = 10, 16, 32
NSEG = 120                  # segments per psum group = 12 centers x 10 bins
GSEG = 128                  # one-hot width (8 pad cols -> FWL weight loads)
NG = 2 * MLOC               # 64 groups/core (2 per molecule slot)
NBLK = 1                    # angular emission blocks
PGRP = 16                   # psum groups packed per PSUM bank tile
RGRP = MLOC // 4            # 8 radial groups of 4 molecules (96 = 4*24 rows)

_TRIU = np.zeros((4, 4), np.int64)
_s1, _s2 = np.triu_indices(4)
_TRIU[_s1, _s2] = np.arange(len(_s1))
_TRIU[_s2, _s1] = _TRIU[_s1, _s2]

# ---- degree-4 (in v=u^2) Chebyshev fit of cos(pi*u/2) on u in [0,1] ----
def _cos_poly():
    v = np.linspace(0.0, 1.0, 4001)
    tgt = np.cos(0.5 * np.pi * np.sqrt(v))
    from numpy.polynomial import chebyshev as C
    ch = C.Chebyshev.fit(v, tgt, 4, domain=[0, 1])
    pw = ch.convert(kind=np.polynomial.Polynomial)
    c = pw.coef  # c0..c4 in v
    K = c[4]
    a = c[:4] / K  # monic residual coeffs a0..a3
    err = np.abs(np.polyval(c[::-1], v) - tgt).max()
    return K, a, err

_POLY_K, _POLY_A, _POLY_ERR = _cos_poly()

# const tile column map ([128, 60] fp32)
_C_SHF2A = 0     # 4  : 2*shf_a
_C_SHFR = 4      # 16 : shf_r
_C_CZH = 20      # 8  : 0.5*cos(shf_z)
_C_SZH = 28      # 8  : 0.5*sin(shf_z)
_C_MASK = 36     # 24 : radial i==j mask*100 (valid on partitions 0..95)
_C_F2B = 60      # 1  : angular exp bias ln(2*K^4)
_C_RADB = 61     # 1  : radial exp bias ln(0.25*K^2)
_C_W = 62


def _build_consts():
    ct = np.zeros((128, _C_W), np.float32)
    ct[:, _C_SHF2A:_C_SHF2A + 4] = 2.0 * SHF_A
    ct[:, _C_SHFR:_C_SHFR + 16] = SHF_R
    ct[:, _C_CZH:_C_CZH + 8] = 0.5 * np.cos(SHF_Z)
    ct[:, _C_SZH:_C_SZH + 8] = 0.5 * np.sin(SHF_Z)
    mask = np.zeros((128, 24), np.float32)
    for mb in range(4):
        for j in range(24):
            mask[mb * 24 + j, j] = 100.0
    ct[:, _C_MASK:_C_MASK + 24] = mask
    K = _POLY_K
    ct[:, _C_F2B] = np.log(2.0) + 4.0 * np.log(abs(K))
    ct[:, _C_RADB] = np.log(0.25) + 2.0 * np.log(abs(K))
    return ct


# ============================================================
# host-side indexing prep (no float math enters the output path)
# ============================================================

def _prep(species, coordinates):
    sp = np.asarray(species)
    co = np.asarray(coordinates, np.float32)
    cod = co.astype(np.float64)
    vec = cod[:, None, :, :] - cod[:, :, None, :]       # [m, i, j, 3] = r_j - r_i
    dmat = np.sqrt(np.maximum((vec ** 2).sum(-1), 0.0))
    adj = (dmat <= RCA) & ~np.eye(A, dtype=bool)[None]

    # per-(m, i) neighbor lists and per-half triple counts
    nbrs = [[np.where(adj[m, i])[0] for i in range(A)] for m in range(M)]
    tri_mi = np.array([[len(nbrs[m][i]) * (len(nbrs[m][i]) - 1) // 2
                        for i in range(A)] for m in range(M)], np.int64)
    Th = np.stack([tri_mi[:, :12].sum(1), tri_mi[:, 12:].sum(1)], 1)  # [M, 2]

    # molecule -> (core, slot): sort by total triples, deal rank-groups of 8
    order = np.argsort(-(Th.sum(1)), kind="stable")
    slot2mol = np.empty((NCORES, MLOC), np.int64)
    for s in range(MLOC):
        for c in range(NCORES):
            slot2mol[c, s] = order[s * NCORES + c]

    # chunks per group (uniform across cores)
    cpg = np.empty(NG, np.int64)
    for s in range(MLOC):
        for h in range(2):
            t = Th[slot2mol[:, s], h]
            cpg[2 * s + h] = max(1, int(np.ceil(t.max() / 128.0)))
    nch = int(cpg.sum())

    # flat triple arrays per core
    pj = np.zeros((NCORES, 128, nch, 3), np.float32)
    pk = np.zeros((NCORES, 128, nch, 3), np.float32)
    ci = np.zeros((NCORES, 128, nch, 3), np.float32)
    oh = np.zeros((NCORES, 128, nch, GSEG), ml_dtypes.bfloat16)

    gstart = np.concatenate([[0], np.cumsum(cpg)])
    for c in range(NCORES):
        for s in range(MLOC):
            m = slot2mol[c, s]
            for h in range(2):
                g = 2 * s + h
                base = gstart[g] * 128
                pos = 0
                for u in range(12):
                    i = h * 12 + u
                    nb = nbrs[m][i]
                    if len(nb) < 2:
                        continue
                    jj, kk = np.triu_indices(len(nb), 1)
                    j, k = nb[jj], nb[kk]
                    p = _TRIU[sp[m, j], sp[m, k]]
                    o = np.argsort(p, kind="stable")
                    j, k, p = j[o], k[o], p[o]
                    n = len(j)
                    sl = slice(base + pos, base + pos + n)
                    t_idx = np.arange(base + pos, base + pos + n)
                    chs, ts = t_idx // 128, t_idx % 128
                    pj[c, ts, chs] = co[m, j]
                    pk[c, ts, chs] = co[m, k]
                    ci[c, ts, chs] = np.broadcast_to(co[m, i], (n, 3))
                    oh[c, ts, chs, p * 12 + u] = 1
                    pos += n
                # pad remainder of the group: far-away fake pair -> w == 0,
                # one-hot row all-zero -> contributes nothing anyway
                tot = cpg[g] * 128
                if pos < tot:
                    t_idx = np.arange(base + pos, base + tot)
                    chs, ts = t_idx // 128, t_idx % 128
                    ref = co[m, 0]
                    pj[c, ts, chs] = ref + np.array([50, 0, 0], np.float32)
                    pk[c, ts, chs] = ref + np.array([0, 50, 0], np.float32)
                    ci[c, ts, chs] = ref

    # ---- radial inputs ----
    # rows: (molecule-in-block mb 0..3, atom j 0..23); groups of 4 slots
    rcj = np.zeros((NCORES, RGRP, 96, 3), np.float32)    # coords of atom j
    rcb = np.zeros((NCORES, RGRP, 96, 72), np.float32)   # molecule coords, (c,i)
    rsp = np.zeros((NCORES, RGRP, 96, 16), ml_dtypes.bfloat16)  # block-diag onehot
    for c in range(NCORES):
        for g in range(RGRP):
            for mb in range(4):
                m = slot2mol[c, g * 4 + mb]
                rows = slice(mb * 24, mb * 24 + 24)
                rcj[c, g, rows] = co[m]
                rcb[c, g, rows] = np.broadcast_to(
                    co[m].T.reshape(-1), (24, 72))
                rsp[c, g, np.arange(mb * 24, mb * 24 + 24),
                    mb * 4 + sp[m]] = 1

    meta = dict(nch=nch, cpg=tuple(int(x) for x in cpg), slot2mol=slot2mol)
    arrays = dict(pj=pj, pk=pk, ci=ci, oh=oh, rcj=rcj, rcb=rcb, rsp=rsp)
    return meta, arrays


# ============================================================
# device program
# ============================================================

def _bb(ap, dims, off=0):
    """Build a broadcast/strided view: keep ap's partition dim, replace free
    dims with explicit [step, count] pairs (element units)."""
    return AP(ap.tensor, ap.offset + off,
              [list(ap.ap[0])] + [list(d) for d in dims])


def _build(nch, cpg):
    nc = bacc.Bacc(None, target_bir_lowering=False)
    pj_d = nc.declare_dram_parameter("pj", [128, nch, 3], dt.float32, False)
    pk_d = nc.declare_dram_parameter("pk", [128, nch, 3], dt.float32, False)
    ci_d = nc.declare_dram_parameter("ci", [128, nch, 3], dt.float32, False)
    oh_d = nc.declare_dram_parameter("oh", [128, nch, GSEG], dt.bfloat16, False)
    rcj_d = nc.declare_dram_parameter("rcj", [RGRP, 96, 3], dt.float32, False)
    rcb_d = nc.declare_dram_parameter("rcb", [RGRP, 96, 72], dt.float32, False)
    rsp_d = nc.declare_dram_parameter("rsp", [RGRP, 96, 16], dt.bfloat16, False)
    ct_d = nc.declare_dram_parameter("consts", [128, _C_W], dt.float32, False)
    outa_d = nc.declare_dram_parameter("outa", [GSEG, NG * 32], dt.float32,
                                       True)
    outr_d = nc.declare_dram_parameter("outr", [16, RGRP * 384], dt.float32,
                                       True)

    gstart = [0]
    for g in range(NG):
        gstart.append(gstart[-1] + cpg[g])

    # block partition of the 64 groups
    gpb = NG // NBLK
    K, a = _POLY_K, _POLY_A
    # fold 2*K^4 (w = 2*fc_j*fc_k = 2*(K^2 s4j^2)(K^2 s4k^2)) into f2's exp bias
    F2BIAS = float(np.log(2.0) + 4.0 * np.log(abs(K)))
    # radial: rad = 0.25 * fc * exp(...) ; fc = (K*s4)^2
    RADBIAS = float(np.log(0.25) + 2.0 * np.log(abs(K)))

    with tile.TileContext(nc) as tc:
        with (
            tc.tile_pool(name="const", bufs=1) as cpool,
            tc.tile_pool(name="io", bufs=1) as io,
            tc.tile_pool(name="geo", bufs=1) as geo,
            tc.tile_pool(name="feat", bufs=1) as feat,
            tc.tile_pool(name="stg", bufs=1) as stg,
            tc.tile_pool(name="gp", bufs=3) as gp,
            tc.tile_pool(name="ps", bufs=4, space="PSUM") as ps,
        ):
            CT = cpool.tile([128, _C_W], dt.float32)
            nc.sync.dma_start(CT[:], ct_d[:])

            AZSTG = stg.tile([GSEG, NG * 32], dt.float32)   # angular staging
            RDSTG = stg.tile([16, RGRP * 384], dt.float32)  # radial staging

            V = nc.vector
            S = nc.scalar

            # angular inputs first: the geometry chain is the critical path
            PJL = io.tile([128, 3 * nch], dt.float32, tag="pj")
            PKL = io.tile([128, 3 * nch], dt.float32, tag="pk")
            CIL = io.tile([128, 3 * nch], dt.float32, tag="ci")
            OHL = io.tile([128, GSEG * nch], dt.bfloat16, tag="oh")
            nc.sync.dma_start(
                PJL[:].rearrange("p (n c) -> p n c", c=3), pj_d[:])
            nc.sync.dma_start(
                CIL[:].rearrange("p (n c) -> p n c", c=3), ci_d[:])
            nc.sync.dma_start(
                PKL[:].rearrange("p (n c) -> p n c", c=3), pk_d[:])

            def poly_fc(dist, nb, tag, rc, npart=128):
                """fc up to factor K^2: returns s4 with fc = (K*s4)^2."""
                u = geo.tile([npart, nb], dt.float32, tag=tag + "_u")
                # (d min rc) mult (1/rc)
                V.tensor_scalar(u[:], dist[:], rc, 1.0 / rc, ALU.min,
                                ALU.mult)
                v = geo.tile([npart, nb], dt.float32, tag=tag + "_v")
                V.tensor_tensor(v[:], u[:], u[:], ALU.mult)
                acc = geo.tile([npart, nb], dt.float32, tag=tag + "_acc")
                V.scalar_tensor_tensor(acc[:], v[:], float(a[3]), v[:],
                                       ALU.add, ALU.mult)
                V.scalar_tensor_tensor(acc[:], acc[:], float(a[2]), v[:],
                                       ALU.add, ALU.mult)
                V.scalar_tensor_tensor(acc[:], acc[:], float(a[1]), v[:],
                                       ALU.add, ALU.mult)
                V.tensor_scalar(acc[:], acc[:], float(a[0]), None, ALU.add)
                return acc

            # ---------------- radial (all 8 groups batched) ----------------
            f32, bf16 = dt.float32, dt.bfloat16
            CJ = io.tile([96, RGRP * 3], f32, tag="rcj")
            CB = io.tile([96, RGRP * 72], f32, tag="rcb")
            SP = io.tile([96, RGRP * 16], bf16, tag="rsp")
            nc.sync.dma_start(CJ[:].rearrange("p (g x) -> p g x", x=3),
                              rcj_d[:].rearrange("g p x -> p g x"))
            nc.sync.dma_start(CB[:].rearrange("p (g x) -> p g x", x=72),
                              rcb_d[:].rearrange("g p x -> p g x"))
            nc.sync.dma_start(SP[:].rearrange("p (g x) -> p g x", x=16),
                              rsp_d[:].rearrange("g p x -> p g x"))
            nc.sync.dma_start(
                OHL[:].rearrange("p (n s) -> p n s", s=GSEG), oh_d[:])

            rv = geo.tile([96, RGRP * 72], f32, tag="rv")
            V.tensor_tensor(
                _bb(rv[:], [[72, RGRP], [24, 3], [1, 24]]),
                _bb(CJ[:], [[3, RGRP], [1, 3], [0, 24]]),
                _bb(CB[:], [[72, RGRP], [24, 3], [1, 24]]),
                ALU.subtract)
            V.tensor_tensor(rv[:], rv[:], rv[:], ALU.mult)
            rd2 = geo.tile([96, RGRP * 24], f32, tag="rd2")
            V.tensor_tensor(rd2[:],
                            _bb(rv[:], [[72, RGRP], [1, 24]], off=0),
                            _bb(rv[:], [[72, RGRP], [1, 24]], off=24),
                            ALU.add)
            V.tensor_tensor(rd2[:], rd2[:],
                            _bb(rv[:], [[72, RGRP], [1, 24]], off=48),
                            ALU.add)
            V.tensor_tensor(rd2[:], rd2[:],
                            _bb(CT[:96, _C_MASK:], [[0, RGRP], [1, 24]]),
                            ALU.add)
            rdist = geo.tile([96, RGRP * 24], f32, tag="rdist")
            S.activation(rdist[:], rd2[:], AF.Ln)
            S.activation(rdist[:], rdist[:], AF.Exp, scale=0.5)

            rs4 = poly_fc(rdist, RGRP * 24, "rf", RCR, npart=96)
            rfc = geo.tile([96, RGRP * 24], f32, tag="rfc")
            V.tensor_tensor(rfc[:], rs4[:], rs4[:], ALU.mult)

            rt = feat.tile([96, RGRP * 384], f32, tag="rt")
            V.tensor_tensor(
                _bb(rt[:], [[384, RGRP], [16, 24], [1, 16]]),
                _bb(rdist[:], [[24, RGRP], [1, 24], [0, 16]]),
                _bb(CT[:96, _C_SHFR:], [[0, RGRP], [0, 24], [1, 16]]),
                ALU.subtract)
            rt2 = feat.tile([96, RGRP * 384], f32, tag="rt2")
            S.activation(rt2[:], rt[:], AF.Square)
            rex = feat.tile([96, RGRP * 384], f32, tag="rex")
            S.activation(rex[:], rt2[:], AF.Exp, scale=-ETA_R,
                         bias=CT[:96, _C_RADB:_C_RADB + 1])
            rad = feat.tile([96, RGRP * 384], bf16, tag="rad")
            V.tensor_tensor(
                _bb(rad[:], [[384, RGRP], [16, 24], [1, 16]]),
                _bb(rfc[:], [[24, RGRP], [1, 24], [0, 16]]),
                _bb(rex[:], [[384, RGRP], [16, 24], [1, 16]]),
                ALU.mult)

            for g in range(RGRP):
                rpt = ps.tile([16, 384], dt.float32, tag="rps")
                nc.tensor.matmul(rpt[:], SP[:, 16 * g:16 * (g + 1)],
                                 rad[:, 384 * g:384 * (g + 1)],
                                 start=True, stop=True)
                sl = slice(384 * g, 384 * (g + 1))
                S.activation(RDSTG[:, sl], rpt[:], AF.Copy)
                nc.sync.dma_start(outr_d[:, sl], RDSTG[:, sl])



            for b in range(NBLK):
                g0, g1 = b * gpb, (b + 1) * gpb
                c0, c1 = gstart[g0], gstart[g1]
                nb = c1 - c0
                f32, bf16 = dt.float32, dt.bfloat16

                PJ, PK, CI, OH = PJL, PKL, CIL, OHL

                vj = geo.tile([128, 3 * nb], f32, tag="vj")
                vk = geo.tile([128, 3 * nb], f32, tag="vk")
                V.tensor_tensor(vj[:], PJ[:], CI[:], ALU.subtract)
                V.tensor_tensor(vk[:], PK[:], CI[:], ALU.subtract)

                sq = geo.tile([128, 3 * nb], f32, tag="sq")
                d2j = geo.tile([128, nb], f32, tag="d2j")
                d2k = geo.tile([128, nb], f32, tag="d2k")
                dot = geo.tile([128, nb], f32, tag="dot")
                def cplane(t, cc):
                    return _bb(t[:], [[3, nb]], off=cc)
                V.tensor_tensor(sq[:], vj[:], vj[:], ALU.mult)
                V.tensor_tensor(d2j[:], cplane(sq, 0), cplane(sq, 1), ALU.add)
                V.tensor_tensor(d2j[:], d2j[:], cplane(sq, 2), ALU.add)
                V.tensor_tensor(sq[:], vk[:], vk[:], ALU.mult)
                V.tensor_tensor(d2k[:], cplane(sq, 0), cplane(sq, 1), ALU.add)
                V.tensor_tensor(d2k[:], d2k[:], cplane(sq, 2), ALU.add)
                V.tensor_tensor(sq[:], vj[:], vk[:], ALU.mult)
                V.tensor_tensor(dot[:], cplane(sq, 0), cplane(sq, 1), ALU.add)
                V.tensor_tensor(dot[:], dot[:], cplane(sq, 2), ALU.add)

                # d, 1/d via single ln + two exps (one ACT table set total)
                lj = geo.tile([128, nb], f32, tag="lj")
                lk = geo.tile([128, nb], f32, tag="lk")
                dj = geo.tile([128, nb], f32, tag="dj")
                dk = geo.tile([128, nb], f32, tag="dk")
                rj = geo.tile([128, nb], f32, tag="rj")
                rk = geo.tile([128, nb], f32, tag="rk")
                S.activation(lj[:], d2j[:], AF.Ln)
                S.activation(lk[:], d2k[:], AF.Ln)
                S.activation(dj[:], lj[:], AF.Exp, scale=0.5)
                S.activation(dk[:], lk[:], AF.Exp, scale=0.5)
                S.activation(rj[:], lj[:], AF.Exp, scale=-0.5)
                S.activation(rk[:], lk[:], AF.Exp, scale=-0.5)

                cos = geo.tile([128, nb], f32, tag="cos")
                V.scalar_tensor_tensor(cos[:], dot[:], 0.95, rj[:],
                                       ALU.mult, ALU.mult)
                V.tensor_tensor(cos[:], cos[:], rk[:], ALU.mult)
                s2 = geo.tile([128, nb], f32, tag="s2")
                V.tensor_tensor(s2[:], cos[:], cos[:], ALU.mult)
                V.tensor_scalar(s2[:], s2[:], -1.0, 1.0, ALU.mult, ALU.add)
                sin = geo.tile([128, nb], f32, tag="sin")
                S.activation(sin[:], s2[:], AF.Ln)
                S.activation(sin[:], sin[:], AF.Exp, scale=0.5)

                s4j = poly_fc(dj, nb, "fj", RCA)
                s4k = poly_fc(dk, nb, "fk", RCA)
                w2 = geo.tile([128, nb], f32, tag="w2")
                V.tensor_tensor(w2[:], s4j[:], s4k[:], ALU.mult)
                V.tensor_tensor(w2[:], w2[:], w2[:], ALU.mult)

                usum = geo.tile([128, nb], f32, tag="usum")
                V.tensor_tensor(usum[:], dj[:], dk[:], ALU.add)

                # f2[a] = exp(-eta/4*(u - 2shf_a)^2 + F2BIAS), layout (n, a)
                t4 = feat.tile([128, 4 * nb], f32, tag="t4")
                V.tensor_tensor(
                    _bb(t4[:], [[4, nb], [1, 4]]),
                    _bb(usum[:], [[1, nb], [0, 4]]),
                    _bb(CT[:, _C_SHF2A:], [[0, nb], [1, 4]]),
                    ALU.subtract)
                t4s = feat.tile([128, 4 * nb], f32, tag="t4s")
                S.activation(t4s[:], t4[:], AF.Square)
                f2 = feat.tile([128, 4 * nb], f32, tag="f2")
                S.activation(f2[:], t4s[:], AF.Exp, scale=-ETA_A / 4.0,
                             bias=CT[:, _C_F2B:_C_F2B + 1])
                wf2 = feat.tile([128, 4 * nb], f32, tag="wf2")
                V.tensor_tensor(
                    _bb(wf2[:], [[4, nb], [1, 4]]),
                    _bb(w2[:], [[1, nb], [0, 4]]),
                    _bb(f2[:], [[4, nb], [1, 4]]),
                    ALU.mult)

                # f1[z] = ((1 + cos(theta - shf_z))/2)^zeta, layout (n, z)
                q8 = feat.tile([128, 8 * nb], f32, tag="q8")
                t8 = feat.tile([128, 8 * nb], f32, tag="t8")
                V.tensor_tensor(
                    _bb(t8[:], [[8, nb], [1, 8]]),
                    _bb(cos[:], [[1, nb], [0, 8]]),
                    _bb(CT[:, _C_CZH:], [[0, nb], [1, 8]]),
                    ALU.mult)
                V.tensor_tensor(
                    _bb(q8[:], [[8, nb], [1, 8]]),
                    _bb(sin[:], [[1, nb], [0, 8]]),
                    _bb(CT[:, _C_SZH:], [[0, nb], [1, 8]]),
                    ALU.mult)
                V.scalar_tensor_tensor(q8[:], t8[:], 0.5, q8[:],
                                       ALU.add, ALU.add)
                S.activation(q8[:], q8[:], AF.Ln)
                f1 = feat.tile([128, 8 * nb], f32, tag="f1")
                S.activation(f1[:], q8[:], AF.Exp, scale=float(ZETA))

                # G[n, a, z] = wf2[n, a] * f1[n, z]   (bf16), one tile
                # per psum group so PE/copies/DMA trail the DVE slice-wise
                gwmax = max(gstart[min(gt + PGRP, g1)] - gstart[gt]
                            for gt in range(g0, g1, PGRP))
                for gt in range(g0, g1, PGRP):
                    ca, cb = gstart[gt] - c0, gstart[min(gt + PGRP, g1)] - c0
                    Gt = gp.tile([128, 32 * gwmax], bf16, tag="G")
                    V.tensor_tensor(
                        _bb(Gt[:], [[32, cb - ca], [8, 4], [1, 8]]),
                        _bb(wf2[:, 4 * ca:], [[4, cb - ca], [1, 4], [0, 8]]),
                        _bb(f1[:, 8 * ca:], [[8, cb - ca], [0, 4], [1, 8]]),
                        ALU.mult)
                    pt = ps.tile([GSEG, 32 * PGRP], dt.float32, tag="ps")
                    for g in range(gt, min(gt + PGRP, g1)):
                        gi = g - gt
                        for k in range(cpg[g]):
                            cl = gstart[g] - gstart[gt] + k
                            nc.tensor.matmul(
                                pt[:, 32 * gi:32 * (gi + 1)],
                                OH[:, GSEG * (gstart[g] - c0 + k):
                                      GSEG * (gstart[g] - c0 + k + 1)],
                                Gt[:, 32 * cl:32 * (cl + 1)],
                                start=(k == 0), stop=(k == cpg[g] - 1))
                    gbase = gt
                    sl = slice(32 * gbase, 32 * (gbase + PGRP))
                    S.activation(AZSTG[:, sl], pt[:], AF.Copy)
                    nc.sync.dma_start(outa_d[:, sl], AZSTG[:, sl])

    _patch_act_tables()
    nc.compile()
    return nc


_ACT_PATCHED = False


def _patch_act_tables():
    """Make Ln/Exp resolve only to the combined natural_log_exp set, so the
    table-load pass emits ONE load instead of thrashing between the ln-only
    and exp-only sets (1.28us per reload)."""
    global _ACT_PATCHED
    if _ACT_PATCHED:
        return
    orig = bacc.get_activation_tables

    def patched(arch):
        t = dict(orig(arch))
        out = {}
        for name, fns in t.items():
            if name != "natural_log_exp_and_others":
                fns = {f for f in fns if f not in (AF.Ln, AF.Exp)}
            out[name] = fns
        return out

    bacc.get_activation_tables = patched
    _ACT_PATCHED = True


_CACHE = {}


def kernel(species, coordinates, coefficients=None):
    species = np.asarray(species)
    coordinates = np.asarray(coordinates, np.float32)
    meta, arrays = _prep(species, coordinates)
    key = (meta["nch"], meta["cpg"])
    if key not in _CACHE:
        _CACHE[key] = _build(meta["nch"], list(meta["cpg"]))
    nc = _CACHE[key]

    ct = _build_consts()
    in_maps = []
    for c in range(NCORES):
        in_maps.append({
            "pj": arrays["pj"][c], "pk": arrays["pk"][c],
            "ci": arrays["ci"][c], "oh": arrays["oh"][c],
            "rcj": arrays["rcj"][c], "rcb": arrays["rcb"][c],
            "rsp": arrays["rsp"][c], "consts": ct,
        })
    res = run_bass_kernel_spmd(nc, in_maps, core_ids=list(range(NCORES)))
    out = np.empty((M, A, 384), np.float32)
    for c in range(NCORES):
        outa = np.asarray(res.results[c]["outa"])  # [128, NG*32]
        outr = np.asarray(res.results[c]["outr"])  # [16, RGRP*384]
        ang = outa.reshape(GSEG, NG, 32)[:120]
        ang = ang.reshape(10, 12, MLOC, 2, 32)          # [p, u, s, h, az]
        ang = ang.transpose(2, 3, 1, 0, 4).reshape(MLOC, A, 320)
        rad = outr.reshape(4, 4, RGRP, A, 16)           # [mb, sp, g, i, r]
        rad = rad.transpose(2, 0, 3, 1, 4).reshape(MLOC, A, 64)
        out[meta["slot2mol"][c], :, :64] = rad
        out[meta["slot2mol"][c], :, 64:] = ang
    return out
